# revision 17
# baseline (speedup 1.0000x reference)
"""DrugRNANet Trainium2 kernel (8 NeuronCores, SPMD).

Sharding: core d owns nodes [32768*d, 32768*(d+1)) and graphs
[512*d, 512*(d+1)).  GCN layers compute hw = h @ W locally, AllGather hw,
then aggregate messages with dma_gather (per-source-shard int16 row
gathers) + one-hot matmuls accumulating in PSUM (self-loop folded in as a
9th local chunk per 128-node block).  Everything downstream is computed
feature-major ([feat, batch]); partition-axis reductions (softmax
denominator, LayerNorm stats) use ones-matmuls on the tensor engine.  The
[B,B] cross-graph attention shards query rows; tm (keys/values source) is
AllGather'ed (tiny).
"""
import sys
sys.path.insert(0, "/opt/trn_rl_repo")

import numpy as np

N_NODES = 262144
N_GRAPHS = 4096
NPG = 64
L = 64
HID = 128
NH = 8
HD = 16
EPS = 1e-5
NCORES = 8
NLOC = N_NODES // NCORES          # 32768 nodes per core
BLOC = NLOC // 128                # 256 blocks per core
SBB = 4                           # blocks per superblock
NSB = BLOC // SBB                 # 64 superblocks
CELL = 128                        # edge slots per (block, srcdev) cell
BLOC_G = N_GRAPHS // NCORES       # 512 graphs per core
LPAD = L + 6                      # padded seq len (70)
IDXCOLS = NSB * NCORES * SBB * CELL // 16   # gidx columns (total idx / 16)

_CACHE = {}


# --------------------------------------------------------------------------
# host-side preprocessing
# --------------------------------------------------------------------------

def _prep(x, edge_index, batch, target, params, dt_np):
    f32 = np.float32
    src = np.asarray(edge_index[0], np.int64)
    dst = np.asarray(edge_index[1], np.int64)
    deg = np.bincount(dst, minlength=N_NODES).astype(f32) + 1.0
    dinv = (1.0 / np.sqrt(deg)).astype(f32)
    ecoef = (dinv[src] * dinv[dst]).astype(f32)
    dinv2 = (dinv * dinv).astype(f32)

    p = params
    shared = {}
    shared["W1"] = np.ascontiguousarray(np.asarray(p["gcn1_W"], f32))
    for i in (2, 3, 4):
        shared[f"W{i}"] = np.ascontiguousarray(
            np.asarray(p[f"gcn{i}_W"], f32)).astype(dt_np)
    for i in (1, 2, 3, 4):
        shared[f"b{i}"] = np.ascontiguousarray(
            np.asarray(p[f"gcn{i}_b"], f32).reshape(HID, 1))
    shared["Wdrug"] = np.ascontiguousarray(np.asarray(p["drug_fc_W"], f32).T)
    shared["bdrug"] = np.asarray(p["drug_fc_b"], f32).reshape(HID, 1).copy()
    for name, k in (("mconv1", 3), ("mconv2", 5), ("mconv3", 7), ("mres", 1)):
        w = np.asarray(p[name + "_W"], f32)          # [128, 5, K]
        shared["w_" + name] = np.ascontiguousarray(
            w.transpose(1, 2, 0).reshape(5 * k, HID))
        shared["b_" + name] = np.asarray(p[name + "_b"], f32).reshape(HID, 1).copy()
    wf = np.asarray(p["mfus_W"], f32)[:, :, 0]       # [128, 384]
    for j in range(3):
        shared[f"w_mfus{j}"] = np.ascontiguousarray(wf[:, j * HID:(j + 1) * HID].T)
    shared["b_mfus"] = np.asarray(p["mfus_b"], f32).reshape(HID, 1).copy()
    shared["Wmfc"] = np.ascontiguousarray(np.asarray(p["mfc_W"], f32).T)
    shared["bmfc"] = np.asarray(p["mfc_b"], f32).reshape(HID, 1).copy()
    wi = np.asarray(p["attn_in_W"], f32)             # [384, 128]
    bi = np.asarray(p["attn_in_b"], f32)
    shared["Wq"] = np.ascontiguousarray(wi[:HID].T)
    shared["Wk"] = np.ascontiguousarray(wi[HID:2 * HID].T)
    shared["Wv"] = np.ascontiguousarray(wi[2 * HID:].T)
    shared["bq"] = bi[:HID].reshape(HID, 1).copy()
    shared["bk"] = bi[HID:2 * HID].reshape(HID, 1).copy()
    shared["bv"] = bi[2 * HID:].reshape(HID, 1).copy()
    wo_t = np.asarray(p["attn_out_W"], f32).T        # [128 in, 128 out]
    shared["WoT2"] = np.ascontiguousarray(
        wo_t.reshape(8, 16, HID).transpose(1, 0, 2).reshape(16, 8 * HID)
    ).astype(np.float16)
    shared["bo"] = np.asarray(p["attn_out_b"], f32).reshape(HID, 1).copy()
    shared["ln_g"] = np.asarray(p["ln_g"], f32).reshape(HID, 1).copy()
    shared["ln_b"] = np.asarray(p["ln_b"], f32).reshape(HID, 1).copy()
    w = np.asarray(p["fus1_W"], f32)                 # [128, 256]
    shared["Wfus1a"] = np.ascontiguousarray(w[:, :HID].T)
    shared["Wfus1b"] = np.ascontiguousarray(w[:, HID:].T)
    shared["bfus1"] = np.asarray(p["fus1_b"], f32).reshape(HID, 1).copy()
    shared["Wfus2"] = np.ascontiguousarray(np.asarray(p["fus2_W"], f32).T)
    shared["bfus2"] = np.asarray(p["fus2_b"], f32).reshape(HID, 1).copy()
    shared["Wcls1"] = np.ascontiguousarray(np.asarray(p["cls1_W"], f32).T)
    shared["bcls1"] = np.asarray(p["cls1_b"], f32).reshape(64, 1).copy()
    shared["Wcls2"] = np.ascontiguousarray(np.asarray(p["cls2_W"], f32).T)
    shared["bcls2"] = np.asarray(p["cls2_b"], f32).reshape(2, 1).copy()

    e16x = np.zeros((17, 16), f32)
    e16x[16, :] = 1.0
    shared["e16x"] = e16x

    x = np.asarray(x, f32)
    target = np.asarray(target, f32)

    in_maps = []
    dstdev = dst >> 15
    for d in range(NCORES):
        m = dstdev == d
        s_d = src[m]
        d_d = dst[m] - d * NLOC
        c_d = ecoef[m]
        blk = d_d >> 7
        sdev = s_d >> 15
        srcloc = (s_d & (NLOC - 1)).astype(np.int64)
        cell = blk * NCORES + sdev
        order = np.argsort(cell, kind="stable")
        cell_s = cell[order]
        cnt = np.bincount(cell_s, minlength=BLOC * NCORES)
        if cnt.max() > CELL:
            raise RuntimeError(f"cell overflow: {cnt.max()} > {CELL}")
        offs = np.zeros(BLOC * NCORES, np.int64)
        offs[1:] = np.cumsum(cnt)[:-1]
        rank = np.arange(len(cell_s)) - offs[cell_s]
        slot = cell_s * CELL + rank
        sidx = np.zeros(BLOC * NCORES * CELL, np.int16)
        scoef = np.zeros(BLOC * NCORES * CELL, f32)
        sdst = np.zeros(BLOC * NCORES * CELL, f32)
        sidx[slot] = srcloc[order].astype(np.int16)
        scoef[slot] = c_d[order]
        sdst[slot] = (d_d[order] & 127).astype(f32)

        sidx3 = sidx.reshape(BLOC, NCORES, CELL)     # [nb, s, p]
        scoef3 = scoef.reshape(BLOC, NCORES, CELL)
        sdst3 = sdst.reshape(BLOC, NCORES, CELL)

        # gather index stream: call (sb, s) covers blocks sb*SBB..+SBB
        gi = sidx3.reshape(NSB, SBB, NCORES, CELL).transpose(0, 2, 1, 3).ravel()
        wrapped = gi.reshape(-1, 16).T               # [16, IDXCOLS]
        gidx = np.ascontiguousarray(np.tile(wrapped, (8, 1)))

        dstloc = np.zeros((CELL, BLOC, 9), f32)
        coefa = np.zeros((CELL, BLOC, 9), f32)
        dstloc[:, :, :8] = sdst3.transpose(2, 0, 1)
        coefa[:, :, :8] = scoef3.transpose(2, 0, 1)
        dstloc[:, :, 8] = np.arange(CELL, dtype=f32)[:, None]
        coefa[:, :, 8] = dinv2[d * NLOC:(d + 1) * NLOC].reshape(BLOC, CELL).T
        dstloc = np.ascontiguousarray(dstloc.reshape(CELL, BLOC * 9))
        coefa = np.ascontiguousarray(coefa.reshape(CELL, BLOC * 9))

        xT = np.ascontiguousarray(x[d * NLOC:(d + 1) * NLOC].T)  # [4, 32768]
        tl = target[d * BLOC_G:(d + 1) * BLOC_G]     # [512, 64, 5]
        tp = np.zeros((5, BLOC_G, LPAD), f32)
        tp[:, :, 3:3 + L] = tl.transpose(2, 0, 1)
        ims = {}
        for nm, K, sh in (("tim3", 3, 2), ("tim5", 5, 1), ("tim7", 7, 0),
                          ("timres", 1, 3)):
            a = np.zeros((5, K, BLOC_G, L), f32)
            for dk in range(K):
                a[:, dk, :, :] = tp[:, :, sh + dk:sh + dk + L]
            ims[nm] = np.ascontiguousarray(a.reshape(5 * K, BLOC_G * L))

        im = dict(shared)
        im.update(xT=xT, gidx=gidx, dstloc=dstloc, coefa=coefa, **ims)
        in_maps.append(im)
    return in_maps


# --------------------------------------------------------------------------
# device program
# --------------------------------------------------------------------------

def _build(dt_h_name):
    import concourse.bass as bass
    import concourse.bacc as bacc
    import concourse.tile as tile
    import concourse.mybir as mybir

    f32 = mybir.dt.float32
    f16 = mybir.dt.float16
    DT = f32 if dt_h_name == "f32" else f16
    AF = mybir.ActivationFunctionType
    AL = mybir.AluOpType

    nc = bacc.Bacc("TRN2", target_bir_lowering=False, debug=False,
                   num_devices=NCORES)

    t_xT = nc.dram_tensor("xT", [4, NLOC], f32, kind="ExternalInput")
    t_tim = {}
    for nm, K in (("tim3", 3), ("tim5", 5), ("tim7", 7), ("timres", 1)):
        t_tim[nm] = nc.dram_tensor(nm, [5 * K, BLOC_G * L], f32,
                                   kind="ExternalInput")
    t_gidx = nc.dram_tensor("gidx", [128, IDXCOLS], mybir.dt.int16,
                            kind="ExternalInput")
    t_dstloc = nc.dram_tensor("dstloc", [CELL, BLOC * 9], f32,
                              kind="ExternalInput")
    t_coefa = nc.dram_tensor("coefa", [CELL, BLOC * 9], f32,
                             kind="ExternalInput")

    wshapes = {"W1": [4, HID], "Wdrug": [HID, HID], "Wmfc": [HID, HID],
               "w_mconv1": [15, HID], "w_mconv2": [25, HID],
               "w_mconv3": [35, HID], "w_mres": [5, HID],
               "w_mfus0": [HID, HID], "w_mfus1": [HID, HID],
               "w_mfus2": [HID, HID], "Wq": [HID, HID], "Wk": [HID, HID],
               "Wv": [HID, HID], "WoT2": [16, 8 * HID], "Wfus1a": [HID, HID],
               "Wfus1b": [HID, HID], "Wfus2": [HID, HID],
               "Wcls1": [HID, 64], "Wcls2": [64, 2], "e16x": [17, 16],
               "bcls1": [64, 1], "bcls2": [2, 1]}
    wnames = ["W1", "b1", "b2", "b3", "b4", "Wdrug", "bdrug",
              "w_mconv1", "w_mconv2", "w_mconv3", "w_mres",
              "b_mconv1", "b_mconv2", "b_mconv3", "b_mres",
              "w_mfus0", "w_mfus1", "w_mfus2", "b_mfus", "Wmfc", "bmfc",
              "Wq", "Wk", "Wv", "bq", "bk", "bv", "WoT2", "bo",
              "ln_g", "ln_b", "Wfus1a", "Wfus1b", "bfus1", "Wfus2", "bfus2",
              "Wcls1", "bcls1", "Wcls2", "bcls2", "e16x"]
    t_w = {}
    for n in wnames:
        dt_n = f16 if n == "WoT2" else f32
        t_w[n] = nc.dram_tensor(n, wshapes.get(n, [HID, 1]), dt_n,
                                kind="ExternalInput")
    for i in (2, 3, 4):
        t_w[f"W{i}"] = nc.dram_tensor(f"W{i}", [HID, HID], DT,
                                      kind="ExternalInput")

    t_out = nc.dram_tensor("out", [2, BLOC_G], f32, kind="ExternalOutput")

    with tile.TileContext(nc) as tc:
        with tc.tile_pool(name="cst", bufs=1) as cst, \
             tc.tile_pool(name="stage", bufs=2) as stage, \
             tc.tile_pool(name="work", bufs=2) as work, \
             tc.tile_pool(name="ohp", bufs=4) as ohp, \
             tc.tile_pool(name="ps", bufs=4, space="PSUM") as ps, \
             tc.tile_pool(name="psh", bufs=2, space="PSUM") as psh, \
             tc.tile_pool(name="dram", bufs=1, space="DRAM") as dram:

            # ---- constants ----
            w = {}
            for n, t in t_w.items():
                wt = cst.tile(list(t.shape), t.dtype, name="w_" + n)
                nc.sync.dma_start(out=wt[:], in_=t[:, :])
                w[n] = wt

            iota_i = cst.tile([128, 128], mybir.dt.int32, name="iota_i")
            nc.gpsimd.iota(iota_i[:], pattern=[[1, 128]], base=0,
                           channel_multiplier=0)
            iota_f = cst.tile([128, 128], f32, name="iota_f")
            nc.vector.tensor_copy(out=iota_f[:], in_=iota_i[:])
            ones128 = cst.tile([128, 1], f32, name="ones128")
            nc.vector.memset(ones128[:], 1.0)
            ones1x = cst.tile([1, 128], f32, name="ones1x")
            nc.vector.memset(ones1x[:], 1.0)
            epsb = cst.tile([1, 1], f32, name="epsb")
            nc.vector.memset(epsb[:], EPS)

            if DT == f16:
                hT = cst.tile([HID, NLOC], DT, name="hT")
                hT_dram = None
            else:
                hT = None
                hT_dram = dram.tile([HID, NLOC], DT, name="hT_dram")

            hw_local = dram.tile([NLOC, HID], DT, name="hw_local")
            hw_fulls = [dram.tile([N_NODES, HID], DT, name=f"hw_full{i}",
                                  addr_space="Shared") for i in (1, 2, 3, 4)]

            # ------------------ GCN layers ------------------
            for layer in (1, 2, 3, 4):
                Wl = w[f"W{layer}"]
                bl = w[f"b{layer}"]

                # hw_local = h @ W (node-major rows)
                for nt4 in range(BLOC // 4):
                    st = stage.tile([128, 4, HID], DT, tag="hwst", name="st")
                    if layer == 1:
                        xld = work.tile([4, 512], f32, tag="xld", name="xld")
                        nc.sync.dma_start(
                            out=xld[:],
                            in_=t_xT[:, nt4 * 512:(nt4 + 1) * 512])
                    for j in range(4):
                        nt = nt4 * 4 + j
                        pt = ps.tile([128, 512], f32, space="PSUM", tag="acc",
                                     name="pt_hw")
                        if layer == 1:
                            nc.tensor.matmul(
                                out=pt[:, :HID],
                                lhsT=xld[:, j * 128:(j + 1) * 128],
                                rhs=w["W1"][:], start=True, stop=True)
                        elif DT == f16:
                            nc.tensor.matmul(
                                out=pt[:, :HID],
                                lhsT=hT[:, nt * 128:(nt + 1) * 128],
                                rhs=Wl[:], start=True, stop=True)
                        else:
                            ht_t = work.tile([HID, 128], DT, tag="htld",
                                             name="ht_t")
                            nc.sync.dma_start(
                                out=ht_t[:],
                                in_=hT_dram[:, nt * 128:(nt + 1) * 128])
                            nc.tensor.matmul(out=pt[:, :HID], lhsT=ht_t[:],
                                             rhs=Wl[:], start=True, stop=True)
                        nc.vector.tensor_copy(out=st[:, j, :], in_=pt[:, :HID])
                    nc.sync.dma_start(
                        out=hw_local[nt4 * 512:(nt4 + 1) * 512, :].rearrange(
                            "(j p) h -> p j h", j=4),
                        in_=st[:])

                hw_full = hw_fulls[layer - 1]
                nc.gpsimd.collective_compute(
                    "AllGather", mybir.AluOpType.bypass,
                    replica_groups=[list(range(NCORES))],
                    ins=[hw_local[:, :]], outs=[hw_full[:, :]],
                )

                # edge aggregation
                ncols_call = SBB * CELL // 16
                for sb in range(NSB):
                    idxt = stage.tile([128, NCORES * ncols_call],
                                      mybir.dt.int16, tag="idxt", name="idxt")
                    nc.sync.dma_start(
                        out=idxt[:],
                        in_=t_gidx[:, sb * NCORES * ncols_call:
                                   (sb + 1) * NCORES * ncols_call])
                    dl = stage.tile([CELL, SBB * 9], f32, tag="dl", name="dl")
                    nc.sync.dma_start(
                        out=dl[:], in_=t_dstloc[:, sb * SBB * 9:(sb + 1) * SBB * 9])
                    cf = stage.tile([CELL, SBB * 9], f32, tag="cf", name="cf")
                    nc.sync.dma_start(
                        out=cf[:], in_=t_coefa[:, sb * SBB * 9:(sb + 1) * SBB * 9])

                    stg = []
                    for s in range(NCORES):
                        g = stage.tile([128, SBB, HID], DT, tag=f"g{s}",
                                       name="g")
                        nc.gpsimd.dma_gather(
                            g[:], hw_full[s * NLOC:(s + 1) * NLOC, :],
                            idxt[:, s * ncols_call:(s + 1) * ncols_call],
                            SBB * CELL, SBB * CELL, HID,
                            single_packet=False,
                        )
                        stg.append(g)
                    gself = stage.tile([128, SBB, HID], DT, tag="gself",
                                       name="gself")
                    nc.sync.dma_start(
                        out=gself[:],
                        in_=hw_local[sb * SBB * 128:(sb + 1) * SBB * 128,
                                     :].rearrange("(i p) h -> p i h", i=SBB))
                    houtst = None
                    if DT != f16:
                        houtst = stage.tile([128, SBB, 128], DT, tag="hout",
                                            name="houtst")
                    for i in range(SBB):
                        nb = sb * SBB + i
                        pt = ps.tile([128, 512], f32, space="PSUM", tag="acc",
                                     name="pt_agg")
                        for c in range(9):
                            col = i * 9 + c
                            oh = ohp.tile([CELL, 128], DT, tag="oh", name="oh")
                            nc.any.tensor_scalar(
                                out=oh[:], in0=iota_f[:],
                                scalar1=dl[:, col:col + 1],
                                scalar2=cf[:, col:col + 1],
                                op0=AL.is_equal, op1=AL.mult,
                            )
                            lhsT = (stg[c][:, i, :] if c < 8 else gself[:, i, :])
                            nc.tensor.matmul(out=pt[:, :128], lhsT=lhsT,
                                             rhs=oh[:],
                                             start=(c == 0), stop=(c == 8))
                        if DT == f16:
                            nc.scalar.activation(
                                out=hT[:, nb * 128:(nb + 1) * 128],
                                in_=pt[:, :128], func=AF.Relu, bias=bl[:])
                        else:
                            nc.scalar.activation(
                                out=houtst[:, i, :], in_=pt[:, :128],
                                func=AF.Relu, bias=bl[:])
                    if DT != f16:
                        nc.sync.dma_start(
                            out=hT_dram[:, sb * SBB * 128:(sb + 1) * SBB * 128],
                            in_=houtst[:])

            # ------------------ pooling + drug fc ------------------
            gT = cst.tile([HID, BLOC_G], f32, name="gT")
            for c4 in range(32):        # 16 graphs per chunk
                if DT == f16:
                    seg = hT[:, c4 * 1024:(c4 + 1) * 1024]
                else:
                    seg_t = work.tile([HID, 1024], DT, tag="poolld",
                                      name="seg_t")
                    nc.sync.dma_start(
                        out=seg_t[:],
                        in_=hT_dram[:, c4 * 1024:(c4 + 1) * 1024])
                    seg = seg_t[:]
                red = work.tile([HID, 16], DT, tag="poolred", name="red")
                nc.vector.tensor_reduce(
                    out=red[:], in_=seg.rearrange("f (g n) -> f g n", n=NPG),
                    axis=mybir.AxisListType.X, op=AL.max)
                nc.vector.tensor_copy(out=gT[:, c4 * 16:(c4 + 1) * 16],
                                      in_=red[:])

            gfcT = cst.tile([HID, BLOC_G], f32, name="gfcT")
            ptd = ps.tile([HID, BLOC_G], f32, space="PSUM", tag="acc",
                          name="ptd")
            nc.tensor.matmul(out=ptd[:], lhsT=w["Wdrug"][:], rhs=gT[:],
                             start=True, stop=True)
            nc.vector.tensor_scalar_add(out=gfcT[:], in0=ptd[:],
                                        scalar1=w["bdrug"][:])

            # ------------------ target conv branch ------------------
            sT = cst.tile([HID, BLOC_G], f32, name="sT")
            convs = [("mconv1", 3, "tim3"), ("mconv2", 5, "tim5"),
                     ("mconv3", 7, "tim7")]
            for bt in range(BLOC_G // 8):
                tks = []
                for name, K, tnm in convs:
                    rhs = work.tile([35, 512], f32, tag="imt", bufs=4,
                                    name="imt")
                    nc.sync.dma_start(
                        out=rhs[:5 * K, :],
                        in_=t_tim[tnm][:, bt * 512:(bt + 1) * 512])
                    ptc = ps.tile([HID, 512], f32, space="PSUM", tag="acc",
                                  name="ptc")
                    nc.tensor.matmul(out=ptc[:], lhsT=w["w_" + name][:],
                                     rhs=rhs[:5 * K, :], start=True, stop=True)
                    tk = work.tile([HID, 512], f32, tag="tk", bufs=4, name="tk")
                    nc.scalar.activation(out=tk[:], in_=ptc[:], func=AF.Relu,
                                         bias=w["b_" + name][:])
                    tks.append(tk)
                ptf = ps.tile([HID, 512], f32, space="PSUM", tag="acc",
                              name="ptf")
                for j in range(3):
                    nc.tensor.matmul(out=ptf[:], lhsT=w[f"w_mfus{j}"][:],
                                     rhs=tks[j][:], start=(j == 0),
                                     stop=(j == 2))
                tf = work.tile([HID, 512], f32, tag="tf", name="tf")
                nc.scalar.activation(out=tf[:], in_=ptf[:], func=AF.Relu,
                                     bias=w["b_mfus"][:])
                tpool = work.tile([HID, 8], f32, tag="tpool", name="tpool")
                nc.vector.tensor_reduce(
                    out=tpool[:], in_=tf[:].rearrange("f (b l) -> f b l", l=64),
                    axis=mybir.AxisListType.X, op=AL.max)
                rres = work.tile([5, 512], f32, tag="imres", name="rres")
                nc.sync.dma_start(
                    out=rres[:],
                    in_=t_tim["timres"][:, bt * 512:(bt + 1) * 512])
                ptr = ps.tile([HID, 512], f32, space="PSUM", tag="acc",
                              name="ptr")
                nc.tensor.matmul(out=ptr[:], lhsT=w["w_mres"][:], rhs=rres[:],
                                 start=True, stop=True)
                tres = work.tile([HID, 8], f32, tag="tres", name="tres")
                nc.vector.tensor_reduce(
                    out=tres[:], in_=ptr[:].rearrange("f (b l) -> f b l", l=64),
                    axis=mybir.AxisListType.X, op=AL.max)
                nc.vector.tensor_scalar_add(out=tres[:], in0=tres[:],
                                            scalar1=w["b_mres"][:])
                tsum = work.tile([HID, 8], f32, tag="tsum", name="tsum")
                nc.vector.tensor_add(out=tsum[:], in0=tpool[:], in1=tres[:])
                nc.vector.tensor_copy(out=sT[:, bt * 8:(bt + 1) * 8],
                                      in_=tsum[:])

            tmT = cst.tile([HID, BLOC_G], f32, name="tmT")
            ptm = ps.tile([HID, BLOC_G], f32, space="PSUM", tag="acc",
                          name="ptm")
            nc.tensor.matmul(out=ptm[:], lhsT=w["Wmfc"][:], rhs=sT[:],
                             start=True, stop=True)
            nc.vector.tensor_scalar_add(out=tmT[:], in0=ptm[:],
                                        scalar1=w["bmfc"][:])

            # ------------------ all-gather tm ------------------
            tm_bounce = dram.tile([HID, BLOC_G], f32, name="tm_bounce")
            tm_gath = dram.tile([NCORES * HID, BLOC_G], f32, name="tm_gath", addr_space="Shared")
            nc.gpsimd.dma_start(out=tm_bounce[:], in_=tmT[:])
            nc.gpsimd.collective_compute(
                "AllGather", mybir.AluOpType.bypass,
                replica_groups=[list(range(NCORES))],
                ins=[tm_bounce[:, :]], outs=[tm_gath[:, :]],
            )
            tmfullT = cst.tile([HID, N_GRAPHS], f32, name="tmfullT")
            nc.sync.dma_start(
                out=tmfullT[:].rearrange("f (c b) -> f c b", c=NCORES),
                in_=tm_gath[:, :].rearrange("(c f) b -> f c b", c=NCORES))

            # ------------------ attention ------------------
            # q/k per-head layouts with heads along the free axis
            qT3 = cst.tile([128, 3 * BLOC_G], f16, name="qT3")
            ptq = ps.tile([HID, BLOC_G], f32, space="PSUM", tag="acc",
                          name="ptq")
            nc.tensor.matmul(out=ptq[:], lhsT=w["Wq"][:], rhs=gfcT[:],
                             start=True, stop=True)
            qT = work.tile([HID, BLOC_G], f32, tag="qT", bufs=1, name="qT")
            nc.vector.tensor_scalar_add(out=qT[:], in0=ptq[:],
                                        scalar1=w["bq"][:])
            for h in range(NH):
                nc.gpsimd.dma_start(
                    out=qT3[(h % 3) * 32:(h % 3) * 32 + HD,
                            (h // 3) * BLOC_G:(h // 3 + 1) * BLOC_G],
                    in_=qT[h * HD:(h + 1) * HD, :])
            kT3 = cst.tile([128, 3 * N_GRAPHS], f16, name="kT3")
            for j in range(NH):
                ptk = ps.tile([HID, 512], f32, space="PSUM", tag="acc",
                              name="ptk")
                nc.tensor.matmul(out=ptk[:], lhsT=w["Wk"][:],
                                 rhs=tmfullT[:, j * 512:(j + 1) * 512],
                                 start=True, stop=True)
                kch = work.tile([HID, 512], f32, tag="kch", name="kch")
                nc.vector.tensor_scalar_add(out=kch[:], in0=ptk[:],
                                            scalar1=w["bk"][:])
                for h in range(NH):
                    nc.gpsimd.dma_start(
                        out=kT3[(h % 3) * 32:(h % 3) * 32 + HD,
                                (h // 3) * N_GRAPHS + j * 512:
                                (h // 3) * N_GRAPHS + (j + 1) * 512],
                        in_=kch[h * HD:(h + 1) * HD, :])
            # v (node-major) with a ones column per head for the denominator
            vt17 = cst.tile([128, 32, NH * 17], f16, name="vt17")
            nc.vector.memset(vt17[:], 1.0)
            for kt in range(32):
                ptv = ps.tile([128, 512], f32, space="PSUM", tag="acc",
                              name="ptv")
                nc.tensor.matmul(out=ptv[:, :HID],
                                 lhsT=tmfullT[:, kt * 128:(kt + 1) * 128],
                                 rhs=w["Wv"][:], start=True, stop=True)
                vch = work.tile([128, HID], f32, tag="vch", name="vch")
                nc.vector.tensor_scalar_add(out=vch[:], in0=ptv[:, :HID],
                                            scalar1=w["bv"][:])
                for h in range(NH):
                    nc.vector.tensor_copy(
                        out=vt17[:, kt, h * 17:h * 17 + 16],
                        in_=vch[:, h * HD:(h + 1) * HD])

            normH = cst.tile([HD, NH * BLOC_G], f16, name="normH")
            for h in range(NH):
                outh = psh.tile([17, BLOC_G], f32, space="PSUM", tag="hold",
                                name="outh")
                for kt in range(32):
                    pts = ps.tile([128, BLOC_G], f32, space="PSUM", tag="acc",
                                  name="pts")
                    nc.tensor.matmul(
                        out=pts[:],
                        lhsT=kT3[(h % 3) * 32:(h % 3) * 32 + HD,
                                 (h // 3) * N_GRAPHS + kt * 128:
                                 (h // 3) * N_GRAPHS + kt * 128 + 128],
                        rhs=qT3[(h % 3) * 32:(h % 3) * 32 + HD,
                                (h // 3) * BLOC_G:(h // 3 + 1) * BLOC_G],
                        start=True, stop=True)
                    pexp = ohp.tile([128, BLOC_G], f16, tag="pexp", bufs=2,
                                    name="pexp")
                    nc.scalar.activation(out=pexp[:], in_=pts[:], func=AF.Exp,
                                         scale=0.25)
                    nc.tensor.matmul(
                        out=outh[:],
                        lhsT=vt17[:, kt, h * 17:(h + 1) * 17], rhs=pexp[:],
                        start=(kt == 0), stop=(kt == 31))
                atth = work.tile([17, BLOC_G], f32, tag="atth", name="atth")
                nc.vector.tensor_copy(out=atth[:], in_=outh[:])
                dbc = ps.tile([128, BLOC_G], f32, space="PSUM", tag="acc",
                              name="dbc")
                nc.tensor.matmul(out=dbc[:HD, :], lhsT=w["e16x"][:],
                                 rhs=atth[:], start=True, stop=True)
                rcp16 = work.tile([HD, BLOC_G], f32, tag="rcp16",
                                  name="rcp16")
                nc.vector.reciprocal(out=rcp16[:], in_=dbc[:HD, :])
                nc.vector.tensor_mul(
                    out=normH[:, h * BLOC_G:(h + 1) * BLOC_G],
                    in0=atth[:16, :], in1=rcp16[:])
            pto = ps.tile([HID, BLOC_G], f32, space="PSUM", tag="acc",
                          name="pto")
            for h in range(NH):
                nc.tensor.matmul(out=pto[:],
                                 lhsT=w["WoT2"][:, h * HID:(h + 1) * HID],
                                 rhs=normH[:, h * BLOC_G:(h + 1) * BLOC_G],
                                 start=(h == 0), stop=(h == NH - 1))
            attp = work.tile([HID, BLOC_G], f32, tag="attp", bufs=1, name="attp")
            nc.vector.tensor_scalar_add(out=attp[:], in0=pto[:],
                                        scalar1=w["bo"][:])

            # ------------------ LayerNorm over feat ------------------
            mu_ps = ps.tile([128, BLOC_G], f32, space="PSUM", tag="acc",
                            name="mu_ps")
            nc.tensor.matmul(out=mu_ps[:1, :], lhsT=ones128[:], rhs=attp[:],
                             start=True, stop=True)
            mu = work.tile([1, BLOC_G], f32, tag="mu", bufs=1, name="mu")
            nc.vector.tensor_scalar_mul(out=mu[:], in0=mu_ps[:1, :],
                                        scalar1=1.0 / HID)
            mu_bc = ps.tile([128, BLOC_G], f32, space="PSUM", tag="acc",
                            name="mu_bc")
            nc.tensor.matmul(out=mu_bc[:], lhsT=ones1x[:], rhs=mu[:],
                             start=True, stop=True)
            cen = work.tile([HID, BLOC_G], f32, tag="cen", bufs=1, name="cen")
            nc.vector.tensor_sub(out=cen[:], in0=attp[:], in1=mu_bc[:])
            sq = work.tile([HID, BLOC_G], f32, tag="sq", bufs=1, name="sq")
            nc.vector.tensor_mul(out=sq[:], in0=cen[:], in1=cen[:])
            var_ps = ps.tile([128, BLOC_G], f32, space="PSUM", tag="acc",
                             name="var_ps")
            nc.tensor.matmul(out=var_ps[:1, :], lhsT=ones128[:], rhs=sq[:],
                             start=True, stop=True)
            sd = work.tile([1, BLOC_G], f32, tag="sd", bufs=1, name="sd")
            nc.scalar.activation(out=sd[:], in_=var_ps[:1, :], func=AF.Sqrt,
                                 scale=1.0 / HID, bias=epsb[:])
            rstd = work.tile([1, BLOC_G], f32, tag="rstd", bufs=1, name="rstd")
            nc.vector.reciprocal(out=rstd[:], in_=sd[:])
            rstd_bc = ps.tile([128, BLOC_G], f32, space="PSUM", tag="acc",
                              name="rstd_bc")
            nc.tensor.matmul(out=rstd_bc[:], lhsT=ones1x[:], rhs=rstd[:],
                             start=True, stop=True)
            nrm = work.tile([HID, BLOC_G], f32, tag="nrm", bufs=1, name="nrm")
            nc.vector.tensor_mul(out=nrm[:], in0=cen[:], in1=rstd_bc[:])
            lnout = work.tile([HID, BLOC_G], f32, tag="lnout", bufs=1, name="lnout")
            nc.vector.tensor_scalar(
                out=lnout[:], in0=nrm[:], scalar1=w["ln_g"][:],
                scalar2=w["ln_b"][:], op0=AL.mult, op1=AL.add)

            # ------------------ fusion + classifier ------------------
            ptf1 = ps.tile([HID, BLOC_G], f32, space="PSUM", tag="acc",
                           name="ptf1")
            nc.tensor.matmul(out=ptf1[:], lhsT=w["Wfus1a"][:], rhs=lnout[:],
                             start=True, stop=False)
            nc.tensor.matmul(out=ptf1[:], lhsT=w["Wfus1b"][:], rhs=tmT[:],
                             start=False, stop=True)
            fus1 = work.tile([HID, BLOC_G], f32, tag="fus1", bufs=1, name="fus1")
            nc.scalar.activation(out=fus1[:], in_=ptf1[:], func=AF.Relu,
                                 bias=w["bfus1"][:])
            ptf2 = ps.tile([HID, BLOC_G], f32, space="PSUM", tag="acc",
                           name="ptf2")
            nc.tensor.matmul(out=ptf2[:], lhsT=w["Wfus2"][:], rhs=fus1[:],
                             start=True, stop=True)
            fus2 = work.tile([HID, BLOC_G], f32, tag="fus2", bufs=1, name="fus2")
            nc.vector.tensor_scalar_add(out=fus2[:], in0=ptf2[:],
                                        scalar1=w["bfus2"][:])
            ptc1 = ps.tile([128, BLOC_G], f32, space="PSUM", tag="acc",
                           name="ptc1")
            nc.tensor.matmul(out=ptc1[:64, :], lhsT=w["Wcls1"][:], rhs=fus2[:],
                             start=True, stop=True)
            c1 = work.tile([64, BLOC_G], f32, tag="c1s", bufs=1, name="c1")
            nc.scalar.activation(out=c1[:], in_=ptc1[:64, :], func=AF.Relu,
                                 bias=w["bcls1"][:])
            ptc2 = ps.tile([128, BLOC_G], f32, space="PSUM", tag="acc",
                           name="ptc2")
            nc.tensor.matmul(out=ptc2[:2, :], lhsT=w["Wcls2"][:], rhs=c1[:],
                             start=True, stop=True)
            c2 = work.tile([2, BLOC_G], f32, tag="c2s", bufs=1, name="c2")
            nc.vector.tensor_scalar_add(out=c2[:], in0=ptc2[:2, :],
                                        scalar1=w["bcls2"][:])
            nc.sync.dma_start(out=t_out[:, :], in_=c2[:])

    nc.compile()
    return nc


def _get_program(dt_h):
    if dt_h not in _CACHE:
        _CACHE[dt_h] = _build(dt_h)
    return _CACHE[dt_h]


def kernel(x, edge_index, batch, target, params, dt_h="f32"):
    from concourse import bass_utils
    np_dt = np.float32 if dt_h == "f32" else np.float16
    in_maps = _prep(x, edge_index, batch, target, params, np_dt)
    nc = _get_program(dt_h)
    res = bass_utils.run_bass_kernel_spmd(nc, in_maps,
                                          core_ids=list(range(NCORES)))
    out = np.zeros((N_GRAPHS, 2), np.float32)
    for d in range(NCORES):
        out[d * BLOC_G:(d + 1) * BLOC_G] = res.results[d]["out"].T
    return out
